# revision 19
# baseline (speedup 1.0000x reference)
"""Trainium2 Bass kernel for nn_BiBayesianConv.

Math (reference):
    delta = 0.5 * log(eps / (1 - eps))                    # [1,F,C,3,3]
    rw    = tanh((weight + delta) / tau)  (tau = 1.0)     # [1,F,C,3,3]
    out[s,b,f,w,h] = sum_{c,k,l} rw[s,f,c,k,l] * x[b,c,w,h]

Since the (k,l) sum is independent of x, we pre-reduce:
    Weff[f,c] = sum_{k,l} tanh(weight[f,c,k,l] + 0.5*(ln eps - ln(1-eps)))
    out[b,f,:] = Weff @ x[b,:,:]          # contraction over C

Sharding: data-parallel over batch. 64 batches / 8 cores = 8 per core.
weight/epsilon replicated; each core computes Weff redundantly (tiny).

Per-core matmul: Weff [F=512, C=256] x x_shard [C=256, N=8*4096].
lhsT layout [C(part), F(free)] obtained by PE-transpose of the
[F(part), C(free)] tiles the elementwise stage naturally produces.
Matmuls run in float32r (full fp32 bits; fast PE mode, 1 cyc/row at
N>=256 vs 4 cyc/row for plain fp32).
"""

import numpy as np

import concourse.bass as bass
import concourse.mybir as mybir
import concourse.tile as tile
from concourse import bacc
from concourse.bass_utils import run_bass_kernel_spmd
from concourse.masks import make_identity

# Problem shapes (hardcoded per contract).
B, C, F = 64, 256, 512
W_SP, H_SP = 64, 64
WH = W_SP * H_SP          # 4096
KL = 9                    # 3*3 kernel taps
N_CORES = 8
B_LOC = B // N_CORES      # 8 batches per core

F32 = mybir.dt.float32
MM_DT = mybir.dt.float32r  # fast-fp32 PE mode; flip to F32 if accuracy demands

P = 128                   # SBUF partitions
CT = C // P               # 2 c-tiles
FT = F // P               # 4 f-tiles
NCHUNK = 512              # matmul moving free dim (one PSUM bank of fp32)
NCH = WH // NCHUNK        # 8 chunks per (b, f-tile)

# Filled by kernel() after each run (BassKernelResults); test harness reads it.
LAST_RESULT = None


def _kernel_body(tc, o_d, x_d, w_d, e_d, b_loc):
    nc = tc.nc
    Ln = mybir.ActivationFunctionType.Ln
    Tanh = mybir.ActivationFunctionType.Tanh
    mult = mybir.AluOpType.mult
    add = mybir.AluOpType.add

    with (
        tc.tile_pool(name="const", bufs=1) as cp,
        tc.tile_pool(name="init", bufs=2) as ip,
        tc.tile_pool(name="xp", bufs=3) as xp,
        tc.tile_pool(name="op", bufs=4) as op,
        tc.tile_pool(name="mmps", bufs=8, space="PSUM") as pp,
    ):
        ident = cp.tile([P, P], F32)
        make_identity(nc, ident)
        # lhsT[ct][ft] [c_part, f_free]: Weff transposed, one tile per 128x128
        # block so each matmul depends only on its own block (compute starts
        # after the first f-tile of the init chain, not the whole thing).
        # dtype float32r: the DVE copy that fills it rounds fp32 -> tf32, which
        # the BIR verifier requires for fp32r matmul inputs.
        lhsT = [[cp.tile([P, P], MM_DT, tag=f"lhsT{ct}_{ft}",
                         name=f"lhsT{ct}_{ft}") for ft in range(FT)]
                for ct in range(CT)]

        # ---- Stage A: Weff = sum_kl tanh(w + 0.5*(ln e - ln(1-e))) ----
        # Emitted per f-tile, interleaved with batch 0's matmul blocks below,
        # so init elementwise work for f-tiles 1-3 queues BEHIND batch 0's
        # PSUM evacuations in the DVE/ACT engine FIFOs (first store ASAP).
        def init_chain(ft):
            fs = slice(ft * P, (ft + 1) * P)
            wt = ip.tile([P, C, KL], F32, tag="wt", name="wt", bufs=1)
            nc.sync.dma_start(out=wt[:], in_=w_d[fs])
            et = ip.tile([P, C, KL], F32, tag="et", name="et", bufs=1)
            nc.sync.dma_start(out=et[:], in_=e_d[fs])

            t1 = ip.tile([P, C, KL], F32, tag="t1", name="t1")
            nc.scalar.activation(out=t1[:], in_=et[:], func=Ln)   # ln(eps)
            # et <- ln(1 - eps)
            nc.scalar.activation(out=et[:], in_=et[:], func=Ln,
                                 scale=-1.0, bias=1.0)
            # t1 <- 0.5*t1 + w
            nc.vector.scalar_tensor_tensor(out=t1[:], in0=t1[:], scalar=0.5,
                                           in1=wt[:], op0=mult, op1=add)
            # t1 <- -0.5*et + t1   (= w + 0.5*(ln e - ln(1-e)))
            nc.vector.scalar_tensor_tensor(out=t1[:], in0=et[:], scalar=-0.5,
                                           in1=t1[:], op0=mult, op1=add)
            nc.scalar.activation(out=t1[:], in_=t1[:], func=Tanh)

            wef = ip.tile([P, C], F32, tag="wef", name="wef", bufs=1)  # [f_part, c]
            nc.vector.tensor_reduce(out=wef[:], in_=t1[:],
                                    axis=mybir.AxisListType.X, op=add)
            for ct in range(CT):
                ps = pp.tile([P, P], F32, tag="mm", name="tps")
                nc.tensor.transpose(ps[:], wef[:, ct * P:(ct + 1) * P],
                                    ident[:])
                nc.vector.tensor_copy(out=lhsT[ct][ft][:], in_=ps[:])

        def load_x(b):
            xt = []
            for ct in range(CT):
                t = xp.tile([P, WH], MM_DT, tag=f"x{ct}", name=f"x{ct}")
                # split loads across both HWDGE rings (sync + scalar) so the
                # early all-load phase runs at 2x single-ring bandwidth
                eng = nc.sync if ct == 0 else nc.scalar
                eng.dma_start(out=t[:], in_=x_d[b, ct * P:(ct + 1) * P])
                xt.append(t)
            return xt

        # ---- Stage B: out[b, f, :] = Weff @ x[b] ----
        # ct-major per (b, ft): the stationary operand changes once per
        # 8-chunk sweep instead of every matmul, and the 8 chunks land in the
        # 8 PSUM banks; evacuation of bank k overlaps the ct=1 sweep.
        def mm_block(b, ft, xt):
            fs = slice(ft * P, (ft + 1) * P)
            ot = op.tile([P, WH], F32, tag="ot", name="ot")
            pss = []
            for ch in range(NCH):
                cs = slice(ch * NCHUNK, (ch + 1) * NCHUNK)
                ps = pp.tile([P, NCHUNK], F32, tag="mm", name=f"mm{ch}")
                nc.tensor.matmul(ps[:], lhsT[0][ft][:], xt[0][:, cs],
                                 start=True, stop=False)
                pss.append(ps)
            for ch in range(NCH):
                cs = slice(ch * NCHUNK, (ch + 1) * NCHUNK)
                nc.tensor.matmul(pss[ch][:], lhsT[1][ft][:], xt[1][:, cs],
                                 start=False, stop=True)
                # balance PSUM evacuation across DVE and ACT (~equal time:
                # ACT copy is ~1.4x slower but ACT has less other work)
                if ch % 2 == 0:
                    nc.vector.tensor_copy(out=ot[:, cs], in_=pss[ch][:])
                else:
                    nc.scalar.copy(out=ot[:, cs], in_=pss[ch][:])
            # split stores across both HWDGE rings; after the loads finish
            # (~2/3 in) the back half would otherwise run on one ring only.
            # Set chosen so the LAST four stores alternate rings 2/2.
            idx = b * FT + ft
            seng = nc.sync if (idx % 16) in (3, 5, 7, 9, 11, 13, 14) else nc.scalar
            seng.dma_start(out=o_d[b, fs], in_=ot[:])

        init_chain(0)
        xt0 = load_x(0)
        for ft in range(FT):
            mm_block(0, ft, xt0)
            if ft + 1 < FT:
                init_chain(ft + 1)
        for b in range(1, b_loc):
            xt = load_x(b)
            for ft in range(FT):
                mm_block(b, ft, xt)


def build_nc(b_loc=B_LOC):
    nc = bacc.Bacc(trn_type="TRN2", target_bir_lowering=False, debug=False)
    x_d = nc.dram_tensor("x", [b_loc, C, WH], MM_DT, kind="ExternalInput").ap()
    w_d = nc.dram_tensor("weight", [F, C, KL], F32, kind="ExternalInput").ap()
    e_d = nc.dram_tensor("epsilon", [F, C, KL], F32, kind="ExternalInput").ap()
    o_d = nc.dram_tensor("out", [b_loc, F, WH], F32, kind="ExternalOutput").ap()
    with tile.TileContext(nc) as tc:
        _kernel_body(tc, o_d, x_d, w_d, e_d, b_loc)
    nc.compile()
    return nc


def kernel(x, weight, epsilon):
    """Full inputs in, full output out. Shards batch across 8 NeuronCores."""
    global LAST_RESULT
    x = np.ascontiguousarray(x, dtype=np.float32).reshape(B, C, WH)
    w = np.ascontiguousarray(weight, dtype=np.float32).reshape(F, C, KL)
    e = np.ascontiguousarray(epsilon, dtype=np.float32).reshape(F, C, KL)

    nc = build_nc()
    in_maps = [
        {"x": x[i * B_LOC:(i + 1) * B_LOC], "weight": w, "epsilon": e}
        for i in range(N_CORES)
    ]
    res = run_bass_kernel_spmd(nc, in_maps, core_ids=list(range(N_CORES)))
    LAST_RESULT = res
    out = np.concatenate(
        [r["out"].reshape(B_LOC, F, W_SP, H_SP) for r in res.results], axis=0
    )
    return out[None]  # [1, B, F, W, H]
